# revision 29
# baseline (speedup 1.0000x reference)
"""Cumulative LayerNorm Trainium2 Bass kernel.

x: [B=8, C=256, T=16000] f32.  Per timestep t: normalize x[:, :, t] by the
mean/std of all elements x[:, :, t'<=t] (cumulative over channels+time), then
scale by weight[c] and add bias[c].

Sharding: pure data parallel over B across 8 NeuronCores (1 sample/core).

Per-core algorithm (C=256 = 2 halves of 128 partitions, T on the free dim):
  Phase A (per 2000-col io-tile):
    - DMA x into SBUF (labeled f32r so the PE may consume it directly).
    - xx = x^2 in bf16 (ACT for half 0, GPSIMD for half 1).
    - PE: s[t] = sum_c x (fp32r, exact ones weights), sq[t] = sum_c x^2
      (bf16) as [2, 500] PSUM rows; evacuate row 0 to SBUF (ACT/DVE copies);
      DMA-reshape rows into a [128, 125] "stat layout" where t = 125*p + i.
  Stats (per 4000-col chunk = 32 stat rows; engine ops need 32-aligned
  partition bases):
    - DVE tensor_tensor_scan along i (per-partition prefix sums).
    - Chunk totals accumulate into st[128, 2]; strict-upper-triangular
      fp32r matmul gives exclusive cross-partition offsets (rows of st for
      future chunks are zeroed so one full-K matmul per chunk is exact).
    - mean = (scan + off) * 1/cnt (off read straight from PSUM);
      var = E[x^2] - mean^2;  istd = exp(-0.5 * ln(var + eps)) on ACT;
      nm = -mean.
  Phase C (per io-tile):
    - Gather istd/nm stat-layout slices back into [1, 2000] rows (DMA).
    - PE: z = I @ x + ones_col x nm_row  (z = x - mean, fp32r identity/ones
      weights);  istd_bc = ones x istd_row, copied PSUM->SBUF on ACT.
    - DVE scalar_tensor_tensor per 500-col block: y = (z * w[p]) * istd_bc,
      then DMA out.
"""

import ml_dtypes
import numpy as np

B, C, T = 8, 256, 16000
P = 128
NH = 2                     # channel halves
CHUNK = 2000               # t per io-tile
NCHUNK = T // CHUNK        # 8
ROWS = T // P              # 125  (stat layout free dim; t = 125*p + i)
PB = 500                   # psum block columns (4 per io-tile)
NPB = CHUNK // PB          # 4
EPS = 1e-06

_cached = {}


def _build_nc(with_bias: bool):
    from contextlib import ExitStack

    import concourse.tile as tile
    from concourse import bacc, mybir

    f32 = mybir.dt.float32
    f32r = mybir.dt.float32r
    bf16 = mybir.dt.bfloat16
    ALU = mybir.AluOpType
    ACTF = mybir.ActivationFunctionType

    nc = bacc.Bacc()

    x = nc.dram_tensor("x", [C, T], f32, kind="ExternalInput")
    wvec = nc.dram_tensor("wvec", [C, 1], f32, kind="ExternalInput")
    iden_d = nc.dram_tensor("iden", [P, P], f32r, kind="ExternalInput")
    tri_d = nc.dram_tensor("tri", [P, P], f32r, kind="ExternalInput")
    ones2r_d = nc.dram_tensor("ones2r", [P, 2], f32r, kind="ExternalInput")
    ones2b_d = nc.dram_tensor("ones2b", [P, 2], bf16, kind="ExternalInput")
    onesb_d = nc.dram_tensor("onesb", [1, P], f32r, kind="ExternalInput")
    zeros2_d = nc.dram_tensor("zeros2", [P, 2], f32r, kind="ExternalInput")
    invcnt_d = nc.dram_tensor("invcnt", [P, ROWS], f32, kind="ExternalInput")
    if with_bias:
        bvec = nc.dram_tensor("bvec", [C, 1], f32, kind="ExternalInput")
    y = nc.dram_tensor("y", [C, T], f32, kind="ExternalOutput")

    with tile.TileContext(nc) as tc, ExitStack() as ctx:
        const = ctx.enter_context(tc.tile_pool(name="const", bufs=1))
        persist = ctx.enter_context(tc.tile_pool(name="persist", bufs=1))
        xpool = ctx.enter_context(tc.tile_pool(name="xpool", bufs=6))
        ypool = ctx.enter_context(tc.tile_pool(name="ypool", bufs=4))
        sqpool = ctx.enter_context(tc.tile_pool(name="sqpool", bufs=2))
        erow = ctx.enter_context(tc.tile_pool(name="erow", bufs=4))
        brow = ctx.enter_context(tc.tile_pool(name="brow", bufs=4))
        ibcsb = ctx.enter_context(tc.tile_pool(name="ibcsb", bufs=4))
        ps_s = ctx.enter_context(tc.tile_pool(name="ps_s", bufs=2, space="PSUM"))
        ps_z = ctx.enter_context(tc.tile_pool(name="ps_z", bufs=4, space="PSUM"))
        ps_i = ctx.enter_context(tc.tile_pool(name="ps_i", bufs=1, space="PSUM"))

        # ---- constants ----
        iden = const.tile([P, P], f32r)
        nc.sync.dma_start(out=iden, in_=iden_d[:, :])
        tri = const.tile([P, P], f32r)
        nc.sync.dma_start(out=tri, in_=tri_d[:, :])
        ones2r = const.tile([P, 2], f32r)
        nc.sync.dma_start(out=ones2r, in_=ones2r_d[:, :])
        ones2b = const.tile([P, 2], bf16)
        nc.sync.dma_start(out=ones2b, in_=ones2b_d[:, :])
        onesb = const.tile([1, P], f32r)
        nc.sync.dma_start(out=onesb, in_=onesb_d[:, :])
        invcnt = const.tile([P, ROWS], f32)
        nc.sync.dma_start(out=invcnt, in_=invcnt_d[:, :])
        w_sb = const.tile([P, NH], f32)
        for h in range(NH):
            nc.sync.dma_start(out=w_sb[:, h : h + 1], in_=wvec[h * P : (h + 1) * P, 0:1])
        if with_bias:
            b_sb = const.tile([P, NH], f32)
            for h in range(NH):
                nc.sync.dma_start(
                    out=b_sb[:, h : h + 1], in_=bvec[h * P : (h + 1) * P, 0:1]
                )
        eps_sb = const.tile([P, 1], f32)
        nc.vector.memset(eps_sb, EPS)

        # ---- persistent stat-layout surfaces ----
        s_re = persist.tile([P, ROWS], f32)     # channel sums -> prefix sums
        sq_re = persist.tile([P, ROWS], f32)
        mean_t = persist.tile([P, ROWS], f32)
        ex2_t = persist.tile([P, ROWS], f32)    # E[x^2] -> var
        msq_t = persist.tile([P, ROWS], f32)    # mean^2 -> ln(var+eps)
        istd_t = persist.tile([P, ROWS], f32)
        nm_t = persist.tile([P, ROWS], f32)     # -mean
        st_sb = persist.tile([P, 2], f32r)      # chunk totals (s, sq)
        nc.sync.dma_start(out=st_sb, in_=zeros2_d[:, :])

        def phase_a(tix, x_t):
            """Load io-tile `tix` (2000 cols), compute channel sums/sumsq into
            stat-layout rows 16*tix .. 16*tix+16."""
            t0 = tix * CHUNK
            for h in range(NH):
                nc.sync.dma_start(
                    out=x_t[:, h, :],
                    in_=x[h * P : (h + 1) * P, t0 : t0 + CHUNK].bitcast(f32r),
                )
            xx0 = sqpool.tile([P, CHUNK], bf16, tag="xx0", name="xx0")
            nc.gpsimd.tensor_tensor(
                xx0, x_t[:, 0, :].bitcast(f32), x_t[:, 0, :].bitcast(f32), ALU.mult
            )
            xx1 = sqpool.tile([P, CHUNK], bf16, tag="xx1", name="xx1")
            nc.gpsimd.tensor_tensor(
                xx1, x_t[:, 1, :].bitcast(f32), x_t[:, 1, :].bitcast(f32), ALU.mult
            )

            for a in range(NPB):
                cs = slice(a * PB, (a + 1) * PB)
                sps = ps_s.tile([2, 512], f32, tag="stat", name="sps")
                nc.tensor.matmul(
                    sps[0:2, 0:PB], ones2r, x_t[:, 0, cs], start=True, stop=False
                )
                nc.tensor.matmul(
                    sps[0:2, 0:PB], ones2r, x_t[:, 1, cs], start=False, stop=True
                )
                qps = ps_s.tile([2, 512], f32, tag="stat", name="qps")
                nc.tensor.matmul(
                    qps[0:2, 0:PB], ones2b, xx0[:, cs], start=True, stop=False
                )
                nc.tensor.matmul(
                    qps[0:2, 0:PB], ones2b, xx1[:, cs], start=False, stop=True
                )
                srow = erow.tile([1, PB], f32, tag="erow", name="srow")
                nc.scalar.copy(srow, sps[0:1, 0:PB])
                qrow = erow.tile([1, PB], f32, tag="erow", name="qrow")
                nc.vector.tensor_copy(qrow, qps[0:1, 0:PB])
                # rows 16*tix+4a .. +4 of the stat layout (t = 125*p + i)
                rp = 16 * tix + 4 * a
                nc.sync.dma_start(out=s_re[rp : rp + 4, :], in_=srow)
                nc.sync.dma_start(out=sq_re[rp : rp + 4, :], in_=qrow)

        def stats(sc):
            """Prefix sums + mean/istd for stat-layout rows 32*sc .. 32*sc+32."""
            sl = slice(32 * sc, 32 * sc + 32)
            nc.vector.tensor_tensor_scan(
                out=s_re[sl, :], data0=s_re[sl, :], data1=s_re[sl, :],
                initial=0.0, op0=ALU.add, op1=ALU.bypass,
            )
            nc.vector.tensor_tensor_scan(
                out=sq_re[sl, :], data0=sq_re[sl, :], data1=sq_re[sl, :],
                initial=0.0, op0=ALU.add, op1=ALU.bypass,
            )
            nc.vector.tensor_copy(st_sb[sl, 0:1], s_re[sl, ROWS - 1 : ROWS])
            nc.vector.tensor_copy(st_sb[sl, 1:2], sq_re[sl, ROWS - 1 : ROWS])
            offps = ps_s.tile([P, 2], f32, tag="stat", name="offps")
            nc.tensor.matmul(offps, tri, st_sb, start=True, stop=True)

            nc.vector.scalar_tensor_tensor(
                out=mean_t[sl, :], in0=s_re[sl, :], scalar=offps[sl, 0:1],
                in1=invcnt[sl, :], op0=ALU.add, op1=ALU.mult,
            )
            nc.vector.scalar_tensor_tensor(
                out=ex2_t[sl, :], in0=sq_re[sl, :], scalar=offps[sl, 1:2],
                in1=invcnt[sl, :], op0=ALU.add, op1=ALU.mult,
            )
            nc.vector.tensor_scalar_mul(nm_t[sl, :], mean_t[sl, :], -1.0)
            nc.vector.tensor_tensor(msq_t[sl, :], mean_t[sl, :], mean_t[sl, :], ALU.mult)
            nc.vector.tensor_tensor(ex2_t[sl, :], ex2_t[sl, :], msq_t[sl, :], ALU.subtract)
            # istd = 1 / sqrt(var + eps)  (Sqrt keeps the ACT table set stable;
            # reciprocal_approx_fast is ~18 bits, far above the fp32r noise)
            nc.scalar.activation(
                msq_t[sl, :], ex2_t[sl, :], ACTF.Sqrt, bias=eps_sb[sl, :], scale=1.0
            )
            nc.vector.reciprocal(out=istd_t[sl, :], in_=msq_t[sl, :])

        def phase_c(tix, x_t):
            """Normalize io-tile `tix` and store it."""
            t0 = tix * CHUNK
            rsl = slice(16 * tix, 16 * tix + 16)
            nm_row = brow.tile([1, CHUNK], f32r, tag="brow", name="nm_row")
            nc.sync.dma_start(out=nm_row, in_=nm_t[rsl, :].bitcast(f32r))
            istd_row = brow.tile([1, CHUNK], f32r, tag="brow", name="istd_row")
            nc.sync.dma_start(out=istd_row, in_=istd_t[rsl, :].bitcast(f32r))

            for half in range(2):  # half-tiles of 1000 columns
                zl = {}
                for h in range(NH):
                    for j in range(2):
                        a = 2 * half + j
                        cs = slice(a * PB, (a + 1) * PB)
                        zps = ps_z.tile([P, 512], f32, tag="z", name="zps")
                        nc.tensor.matmul(
                            zps[:, 0:PB], iden, x_t[:, h, cs], start=True, stop=False
                        )
                        nc.tensor.matmul(
                            zps[:, 0:PB], onesb, nm_row[0:1, cs],
                            start=False, stop=True,
                        )
                        zl[h, j] = zps
                ibc = ps_i.tile([P, 2, 512], f32, tag="ibc_ps", name="ibc")
                for j in range(2):
                    a = 2 * half + j
                    cs = slice(a * PB, (a + 1) * PB)
                    nc.tensor.matmul(
                        ibc[:, j, 0:PB], onesb, istd_row[0:1, cs], start=True, stop=True
                    )
                ibc_sb = ibcsb.tile([P, 2, 512], f32, tag="ibc", name="ibc_sb")
                nc.scalar.copy(ibc_sb[:, :, 0:PB], ibc[:, :, 0:PB])
                for h in range(NH):
                    for j in range(2):
                        a = 2 * half + j
                        y_t = ypool.tile([P, 512], f32, tag="y", name="y_t")
                        nc.vector.scalar_tensor_tensor(
                            out=y_t[:, 0:PB], in0=zl[h, j][:, 0:PB],
                            scalar=w_sb[:, h : h + 1], in1=ibc_sb[:, j, 0:PB],
                            op0=ALU.mult, op1=ALU.mult,
                        )
                        if with_bias:
                            nc.vector.tensor_scalar_add(
                                out=y_t[:, 0:PB], in0=y_t[:, 0:PB],
                                scalar1=b_sb[:, h : h + 1],
                            )
                        nc.sync.dma_start(
                            out=y[h * P : (h + 1) * P,
                                  t0 + a * PB : t0 + (a + 1) * PB],
                            in_=y_t[:, 0:PB],
                        )

        for sc in range(NCHUNK // 2):  # stats chunks of 4000 cols
            tiles = []
            for k in range(2):
                x_t = xpool.tile([P, NH, CHUNK], f32r, tag="x", name="x_t")
                phase_a(2 * sc + k, x_t)
                tiles.append(x_t)
            stats(sc)
            for k in range(2):
                phase_c(2 * sc + k, tiles[k])
    nc.compile()
    return nc


def _consts():
    iden = np.eye(P, dtype=np.float32)
    tri = np.triu(np.ones((P, P), dtype=np.float32), k=1)  # tri[k,m]=1 iff k<m
    ones2 = np.ones((P, 2), dtype=np.float32)
    onesb = np.ones((1, P), dtype=np.float32)
    t_idx = (125 * np.arange(P, dtype=np.float64)[:, None]
             + np.arange(ROWS, dtype=np.float64)[None, :])
    invcnt = (1.0 / (C * (t_idx + 1.0))).astype(np.float32)
    return {"iden": iden, "tri": tri, "ones2r": ones2,
            "ones2b": ones2.astype(ml_dtypes.bfloat16), "onesb": onesb,
            "zeros2": np.zeros((P, 2), dtype=np.float32), "invcnt": invcnt}


def _get_nc(with_bias: bool):
    key = ("nc", with_bias)
    if key not in _cached:
        _cached[key] = _build_nc(with_bias)
    return _cached[key]


def _run(x, weight, bias, trace=False):
    from concourse.bass_utils import run_bass_kernel_spmd

    x = np.ascontiguousarray(np.asarray(x, dtype=np.float32))
    weight = np.asarray(weight, dtype=np.float32).reshape(C, 1)
    bias = np.asarray(bias, dtype=np.float32).reshape(C, 1)
    with_bias = bool(np.any(bias))
    nc = _get_nc(with_bias)

    consts = _consts()
    in_maps = []
    for b in range(B):
        m = {"x": np.ascontiguousarray(x[b]), "wvec": weight}
        if with_bias:
            m["bvec"] = bias
        m.update(consts)
        in_maps.append(m)

    res = run_bass_kernel_spmd(nc, in_maps, core_ids=list(range(B)), trace=trace)
    y = np.stack([r["y"] for r in res.results], axis=0)
    return y, res


def kernel(x, weight, bias):
    y, _ = _run(x, weight, bias, trace=False)
    return y


# revision 30
# speedup vs baseline: 1.0589x; 1.0589x over previous
"""Cumulative LayerNorm Trainium2 Bass kernel.

x: [B=8, C=256, T=16000] f32.  Per timestep t: normalize x[:, :, t] by the
mean/std of all elements x[:, :, t'<=t] (cumulative over channels+time), then
scale by weight[c] and add bias[c].

Sharding: pure data parallel over B across 8 NeuronCores (1 sample/core).

Per-core algorithm (C=256 = 2 halves of 128 partitions, T on the free dim):
  Phase A (per 2000-col io-tile):
    - DMA x into SBUF (labeled f32r so the PE may consume it directly).
    - xx = x^2 in bf16 (ACT for half 0, GPSIMD for half 1).
    - PE: s[t] = sum_c x (fp32r, exact ones weights), sq[t] = sum_c x^2
      (bf16) as [2, 500] PSUM rows; evacuate row 0 to SBUF (ACT/DVE copies);
      DMA-reshape rows into a [128, 125] "stat layout" where t = 125*p + i.
  Stats (per 4000-col chunk = 32 stat rows; engine ops need 32-aligned
  partition bases):
    - DVE tensor_tensor_scan along i (per-partition prefix sums).
    - Chunk totals accumulate into st[128, 2]; strict-upper-triangular
      fp32r matmul gives exclusive cross-partition offsets (rows of st for
      future chunks are zeroed so one full-K matmul per chunk is exact).
    - mean = (scan + off) * 1/cnt (off read straight from PSUM);
      var = E[x^2] - mean^2;  istd = exp(-0.5 * ln(var + eps)) on ACT;
      nm = -mean.
  Phase C (per io-tile):
    - Gather istd/nm stat-layout slices back into [1, 2000] rows (DMA).
    - PE: z = I @ x + ones_col x nm_row  (z = x - mean, fp32r identity/ones
      weights);  istd_bc = ones x istd_row, copied PSUM->SBUF on ACT.
    - DVE scalar_tensor_tensor per 500-col block: y = (z * w[p]) * istd_bc,
      then DMA out.
"""

import ml_dtypes
import numpy as np

B, C, T = 8, 256, 16000
P = 128
NH = 2                     # channel halves
CHUNK = 2000               # t per io-tile
NCHUNK = T // CHUNK        # 8
ROWS = T // P              # 125  (stat layout free dim; t = 125*p + i)
PB = 500                   # psum block columns (4 per io-tile)
NPB = CHUNK // PB          # 4
EPS = 1e-06

_cached = {}


def _build_nc(with_bias: bool):
    from contextlib import ExitStack

    import concourse.tile as tile
    from concourse import bacc, mybir

    f32 = mybir.dt.float32
    f32r = mybir.dt.float32r
    bf16 = mybir.dt.bfloat16
    ALU = mybir.AluOpType
    ACTF = mybir.ActivationFunctionType

    nc = bacc.Bacc()

    x = nc.dram_tensor("x", [C, T], f32, kind="ExternalInput")
    wvec = nc.dram_tensor("wvec", [C, 1], f32, kind="ExternalInput")
    iden_d = nc.dram_tensor("iden", [P, P], f32r, kind="ExternalInput")
    tri_d = nc.dram_tensor("tri", [P, P], f32r, kind="ExternalInput")
    ones2r_d = nc.dram_tensor("ones2r", [P, 2], f32r, kind="ExternalInput")
    ones2b_d = nc.dram_tensor("ones2b", [P, 2], bf16, kind="ExternalInput")
    onesb_d = nc.dram_tensor("onesb", [1, P], f32r, kind="ExternalInput")
    zeros2_d = nc.dram_tensor("zeros2", [P, 2], f32r, kind="ExternalInput")
    invcnt_d = nc.dram_tensor("invcnt", [P, ROWS], f32, kind="ExternalInput")
    if with_bias:
        bvec = nc.dram_tensor("bvec", [C, 1], f32, kind="ExternalInput")
    y = nc.dram_tensor("y", [C, T], f32, kind="ExternalOutput")

    with tile.TileContext(nc) as tc, ExitStack() as ctx:
        const = ctx.enter_context(tc.tile_pool(name="const", bufs=1))
        persist = ctx.enter_context(tc.tile_pool(name="persist", bufs=1))
        xpool = ctx.enter_context(tc.tile_pool(name="xpool", bufs=6))
        ypool = ctx.enter_context(tc.tile_pool(name="ypool", bufs=4))
        sqpool = ctx.enter_context(tc.tile_pool(name="sqpool", bufs=2))
        erow = ctx.enter_context(tc.tile_pool(name="erow", bufs=4))
        brow = ctx.enter_context(tc.tile_pool(name="brow", bufs=4))
        ibcsb = ctx.enter_context(tc.tile_pool(name="ibcsb", bufs=4))
        ps_s = ctx.enter_context(tc.tile_pool(name="ps_s", bufs=2, space="PSUM"))
        ps_z = ctx.enter_context(tc.tile_pool(name="ps_z", bufs=4, space="PSUM"))
        ps_i = ctx.enter_context(tc.tile_pool(name="ps_i", bufs=1, space="PSUM"))

        # ---- constants ----
        iden = const.tile([P, P], f32r)
        nc.sync.dma_start(out=iden, in_=iden_d[:, :])
        tri = const.tile([P, P], f32r)
        nc.sync.dma_start(out=tri, in_=tri_d[:, :])
        ones2r = const.tile([P, 2], f32r)
        nc.sync.dma_start(out=ones2r, in_=ones2r_d[:, :])
        ones2b = const.tile([P, 2], bf16)
        nc.sync.dma_start(out=ones2b, in_=ones2b_d[:, :])
        onesb = const.tile([1, P], f32r)
        nc.sync.dma_start(out=onesb, in_=onesb_d[:, :])
        invcnt = const.tile([P, ROWS], f32)
        nc.sync.dma_start(out=invcnt, in_=invcnt_d[:, :])
        w_sb = const.tile([P, NH], f32)
        for h in range(NH):
            nc.sync.dma_start(out=w_sb[:, h : h + 1], in_=wvec[h * P : (h + 1) * P, 0:1])
        if with_bias:
            b_sb = const.tile([P, NH], f32)
            for h in range(NH):
                nc.sync.dma_start(
                    out=b_sb[:, h : h + 1], in_=bvec[h * P : (h + 1) * P, 0:1]
                )
        eps_sb = const.tile([P, 1], f32)
        nc.vector.memset(eps_sb, EPS)

        # ---- persistent stat-layout surfaces ----
        s_re = persist.tile([P, ROWS], f32)     # channel sums -> prefix sums
        sq_re = persist.tile([P, ROWS], f32)
        mean_t = persist.tile([P, ROWS], f32)
        ex2_t = persist.tile([P, ROWS], f32)    # E[x^2] -> var
        msq_t = persist.tile([P, ROWS], f32)    # mean^2 -> ln(var+eps)
        istd_t = persist.tile([P, ROWS], f32)
        nm_t = persist.tile([P, ROWS], f32)     # -mean
        st_sb = persist.tile([P, 2], f32r)      # chunk totals (s, sq)
        nc.sync.dma_start(out=st_sb, in_=zeros2_d[:, :])

        def phase_a(tix, x_t):
            """Load io-tile `tix` (2000 cols), compute channel sums/sumsq into
            stat-layout rows 16*tix .. 16*tix+16."""
            t0 = tix * CHUNK
            for h in range(NH):
                nc.sync.dma_start(
                    out=x_t[:, h, :],
                    in_=x[h * P : (h + 1) * P, t0 : t0 + CHUNK].bitcast(f32r),
                )
            xx0 = sqpool.tile([P, CHUNK], bf16, tag="xx0", name="xx0")
            nc.scalar.activation(xx0, x_t[:, 0, :].bitcast(f32), ACTF.Square)
            xx1 = sqpool.tile([P, CHUNK], bf16, tag="xx1", name="xx1")
            nc.gpsimd.tensor_tensor(
                xx1, x_t[:, 1, :].bitcast(f32), x_t[:, 1, :].bitcast(f32), ALU.mult
            )

            for a in range(NPB):
                cs = slice(a * PB, (a + 1) * PB)
                sps = ps_s.tile([2, 512], f32, tag="stat", name="sps")
                nc.tensor.matmul(
                    sps[0:2, 0:PB], ones2r, x_t[:, 0, cs], start=True, stop=False
                )
                nc.tensor.matmul(
                    sps[0:2, 0:PB], ones2r, x_t[:, 1, cs], start=False, stop=True
                )
                qps = ps_s.tile([2, 512], f32, tag="stat", name="qps")
                nc.tensor.matmul(
                    qps[0:2, 0:PB], ones2b, xx0[:, cs], start=True, stop=False
                )
                nc.tensor.matmul(
                    qps[0:2, 0:PB], ones2b, xx1[:, cs], start=False, stop=True
                )
                srow = erow.tile([1, PB], f32, tag="erow", name="srow")
                nc.scalar.copy(srow, sps[0:1, 0:PB])
                qrow = erow.tile([1, PB], f32, tag="erow", name="qrow")
                nc.vector.tensor_copy(qrow, qps[0:1, 0:PB])
                # rows 16*tix+4a .. +4 of the stat layout (t = 125*p + i)
                rp = 16 * tix + 4 * a
                nc.sync.dma_start(out=s_re[rp : rp + 4, :], in_=srow)
                nc.sync.dma_start(out=sq_re[rp : rp + 4, :], in_=qrow)

        def stats(sc):
            """Prefix sums + mean/istd for stat-layout rows 32*sc .. 32*sc+32."""
            sl = slice(32 * sc, 32 * sc + 32)
            nc.vector.tensor_tensor_scan(
                out=s_re[sl, :], data0=s_re[sl, :], data1=s_re[sl, :],
                initial=0.0, op0=ALU.add, op1=ALU.bypass,
            )
            nc.vector.tensor_tensor_scan(
                out=sq_re[sl, :], data0=sq_re[sl, :], data1=sq_re[sl, :],
                initial=0.0, op0=ALU.add, op1=ALU.bypass,
            )
            nc.vector.tensor_copy(st_sb[sl, 0:1], s_re[sl, ROWS - 1 : ROWS])
            nc.vector.tensor_copy(st_sb[sl, 1:2], sq_re[sl, ROWS - 1 : ROWS])
            offps = ps_s.tile([P, 2], f32, tag="stat", name="offps")
            nc.tensor.matmul(offps, tri, st_sb, start=True, stop=True)

            nc.vector.scalar_tensor_tensor(
                out=mean_t[sl, :], in0=s_re[sl, :], scalar=offps[sl, 0:1],
                in1=invcnt[sl, :], op0=ALU.add, op1=ALU.mult,
            )
            nc.vector.scalar_tensor_tensor(
                out=ex2_t[sl, :], in0=sq_re[sl, :], scalar=offps[sl, 1:2],
                in1=invcnt[sl, :], op0=ALU.add, op1=ALU.mult,
            )
            nc.vector.tensor_scalar_mul(nm_t[sl, :], mean_t[sl, :], -1.0)
            nc.vector.tensor_tensor(msq_t[sl, :], mean_t[sl, :], mean_t[sl, :], ALU.mult)
            nc.vector.tensor_tensor(ex2_t[sl, :], ex2_t[sl, :], msq_t[sl, :], ALU.subtract)
            # istd = 1 / sqrt(var + eps)  (Sqrt keeps the ACT table set stable;
            # reciprocal_approx_fast is ~18 bits, far above the fp32r noise)
            nc.scalar.activation(
                msq_t[sl, :], ex2_t[sl, :], ACTF.Sqrt, bias=eps_sb[sl, :], scale=1.0
            )
            nc.vector.reciprocal(out=istd_t[sl, :], in_=msq_t[sl, :])

        def phase_c(tix, x_t):
            """Normalize io-tile `tix` and store it."""
            t0 = tix * CHUNK
            rsl = slice(16 * tix, 16 * tix + 16)
            nm_row = brow.tile([1, CHUNK], f32r, tag="brow", name="nm_row")
            nc.sync.dma_start(out=nm_row, in_=nm_t[rsl, :].bitcast(f32r))
            istd_row = brow.tile([1, CHUNK], f32r, tag="brow", name="istd_row")
            nc.sync.dma_start(out=istd_row, in_=istd_t[rsl, :].bitcast(f32r))

            for half in range(2):  # half-tiles of 1000 columns
                zl = {}
                for h in range(NH):
                    for j in range(2):
                        a = 2 * half + j
                        cs = slice(a * PB, (a + 1) * PB)
                        zps = ps_z.tile([P, 512], f32, tag="z", name="zps")
                        nc.tensor.matmul(
                            zps[:, 0:PB], iden, x_t[:, h, cs], start=True, stop=False
                        )
                        nc.tensor.matmul(
                            zps[:, 0:PB], onesb, nm_row[0:1, cs],
                            start=False, stop=True,
                        )
                        zl[h, j] = zps
                ibc = ps_i.tile([P, 2, 512], f32, tag="ibc_ps", name="ibc")
                for j in range(2):
                    a = 2 * half + j
                    cs = slice(a * PB, (a + 1) * PB)
                    nc.tensor.matmul(
                        ibc[:, j, 0:PB], onesb, istd_row[0:1, cs], start=True, stop=True
                    )
                ibc_sb = ibcsb.tile([P, 2, 512], f32, tag="ibc", name="ibc_sb")
                nc.scalar.copy(ibc_sb[:, :, 0:PB], ibc[:, :, 0:PB])
                for h in range(NH):
                    for j in range(2):
                        a = 2 * half + j
                        y_t = ypool.tile([P, 512], f32, tag="y", name="y_t")
                        nc.vector.scalar_tensor_tensor(
                            out=y_t[:, 0:PB], in0=zl[h, j][:, 0:PB],
                            scalar=w_sb[:, h : h + 1], in1=ibc_sb[:, j, 0:PB],
                            op0=ALU.mult, op1=ALU.mult,
                        )
                        if with_bias:
                            nc.vector.tensor_scalar_add(
                                out=y_t[:, 0:PB], in0=y_t[:, 0:PB],
                                scalar1=b_sb[:, h : h + 1],
                            )
                        nc.sync.dma_start(
                            out=y[h * P : (h + 1) * P,
                                  t0 + a * PB : t0 + (a + 1) * PB],
                            in_=y_t[:, 0:PB],
                        )

        for sc in range(NCHUNK // 2):  # stats chunks of 4000 cols
            tiles = []
            for k in range(2):
                x_t = xpool.tile([P, NH, CHUNK], f32r, tag="x", name="x_t")
                phase_a(2 * sc + k, x_t)
                tiles.append(x_t)
            stats(sc)
            for k in range(2):
                phase_c(2 * sc + k, tiles[k])
    nc.compile()
    return nc


def _consts():
    iden = np.eye(P, dtype=np.float32)
    tri = np.triu(np.ones((P, P), dtype=np.float32), k=1)  # tri[k,m]=1 iff k<m
    ones2 = np.ones((P, 2), dtype=np.float32)
    onesb = np.ones((1, P), dtype=np.float32)
    t_idx = (125 * np.arange(P, dtype=np.float64)[:, None]
             + np.arange(ROWS, dtype=np.float64)[None, :])
    invcnt = (1.0 / (C * (t_idx + 1.0))).astype(np.float32)
    return {"iden": iden, "tri": tri, "ones2r": ones2,
            "ones2b": ones2.astype(ml_dtypes.bfloat16), "onesb": onesb,
            "zeros2": np.zeros((P, 2), dtype=np.float32), "invcnt": invcnt}


def _get_nc(with_bias: bool):
    key = ("nc", with_bias)
    if key not in _cached:
        _cached[key] = _build_nc(with_bias)
    return _cached[key]


def _run(x, weight, bias, trace=False):
    from concourse.bass_utils import run_bass_kernel_spmd

    x = np.ascontiguousarray(np.asarray(x, dtype=np.float32))
    weight = np.asarray(weight, dtype=np.float32).reshape(C, 1)
    bias = np.asarray(bias, dtype=np.float32).reshape(C, 1)
    with_bias = bool(np.any(bias))
    nc = _get_nc(with_bias)

    consts = _consts()
    in_maps = []
    for b in range(B):
        m = {"x": np.ascontiguousarray(x[b]), "wvec": weight}
        if with_bias:
            m["bvec"] = bias
        m.update(consts)
        in_maps.append(m)

    res = run_bass_kernel_spmd(nc, in_maps, core_ids=list(range(B)), trace=trace)
    y = np.stack([r["y"] for r in res.results], axis=0)
    return y, res


def kernel(x, weight, bias):
    y, _ = _run(x, weight, bias, trace=False)
    return y


# revision 36
# speedup vs baseline: 1.1251x; 1.0625x over previous
"""Cumulative LayerNorm Trainium2 Bass kernel.

x: [B=8, C=256, T=16000] f32.  Per timestep t: normalize x[:, :, t] by the
mean/std of all elements x[:, :, t'<=t] (cumulative over channels+time), then
scale by weight[c] and add bias[c].

Sharding: pure data parallel over B across 8 NeuronCores (1 sample/core).

Per-core algorithm (C=256 = 2 halves of 128 partitions, T on the free dim):
  Phase A (per 2000-col io-tile):
    - DMA x into SBUF (labeled f32r so the PE may consume it directly).
    - xx = x^2 in bf16 (ACT for half 0, GPSIMD for half 1).
    - PE: s[t] = sum_c x (fp32r, exact ones weights), sq[t] = sum_c x^2
      (bf16) as [2, 500] PSUM rows; evacuate row 0 to SBUF (ACT/DVE copies);
      DMA-reshape rows into a [128, 125] "stat layout" where t = 125*p + i.
  Stats (per 4000-col chunk = 32 stat rows; engine ops need 32-aligned
  partition bases):
    - DVE tensor_tensor_scan along i (per-partition prefix sums).
    - Chunk totals accumulate into st[128, 2]; strict-upper-triangular
      fp32r matmul gives exclusive cross-partition offsets (rows of st for
      future chunks are zeroed so one full-K matmul per chunk is exact).
    - mean = (scan + off) * 1/cnt (off read straight from PSUM);
      var = E[x^2] - mean^2;  istd = exp(-0.5 * ln(var + eps)) on ACT;
      nm = -mean.
  Phase C (per io-tile):
    - Gather istd/nm stat-layout slices back into [1, 2000] rows (DMA).
    - PE: z = I @ x + ones_col x nm_row  (z = x - mean, fp32r identity/ones
      weights);  istd_bc = ones x istd_row, copied PSUM->SBUF on ACT.
    - DVE scalar_tensor_tensor per 500-col block: y = (z * w[p]) * istd_bc,
      then DMA out.
"""

import ml_dtypes
import numpy as np

B, C, T = 8, 256, 16000
P = 128
NH = 2                     # channel halves
CHUNK = 2000               # t per io-tile
NCHUNK = T // CHUNK        # 8
ROWS = T // P              # 125  (stat layout free dim; t = 125*p + i)
PB = 500                   # psum block columns (4 per io-tile)
NPB = CHUNK // PB          # 4
EPS = 1e-06

_cached = {}


def _build_nc(with_bias: bool):
    from contextlib import ExitStack

    import concourse.tile as tile
    from concourse import bacc, mybir

    f32 = mybir.dt.float32
    f32r = mybir.dt.float32r
    bf16 = mybir.dt.bfloat16
    ALU = mybir.AluOpType
    ACTF = mybir.ActivationFunctionType

    nc = bacc.Bacc()

    x = nc.dram_tensor("x", [C, T], f32, kind="ExternalInput")
    wvec = nc.dram_tensor("wvec", [C, 1], f32, kind="ExternalInput")
    iden_d = nc.dram_tensor("iden", [P, P], f32r, kind="ExternalInput")
    tri_d = nc.dram_tensor("tri", [P, P], f32r, kind="ExternalInput")
    ones2r_d = nc.dram_tensor("ones2r", [P, 2], f32r, kind="ExternalInput")
    ones2b_d = nc.dram_tensor("ones2b", [P, 2], bf16, kind="ExternalInput")
    onesb_d = nc.dram_tensor("onesb", [1, P], f32r, kind="ExternalInput")
    zeros2_d = nc.dram_tensor("zeros2", [P, 2], f32r, kind="ExternalInput")
    invcnt_d = nc.dram_tensor("invcnt", [P, ROWS], f32, kind="ExternalInput")
    if with_bias:
        bvec = nc.dram_tensor("bvec", [C, 1], f32, kind="ExternalInput")
    y = nc.dram_tensor("y", [C, T], f32, kind="ExternalOutput")

    with tile.TileContext(nc) as tc, ExitStack() as ctx:
        const = ctx.enter_context(tc.tile_pool(name="const", bufs=1))
        persist = ctx.enter_context(tc.tile_pool(name="persist", bufs=1))
        xpool = ctx.enter_context(tc.tile_pool(name="xpool", bufs=6))
        ypool = ctx.enter_context(tc.tile_pool(name="ypool", bufs=4))
        sqpool = ctx.enter_context(tc.tile_pool(name="sqpool", bufs=2))
        erow = ctx.enter_context(tc.tile_pool(name="erow", bufs=4))
        brow = ctx.enter_context(tc.tile_pool(name="brow", bufs=4))
        ibcsb = ctx.enter_context(tc.tile_pool(name="ibcsb", bufs=4))
        ps_s = ctx.enter_context(tc.tile_pool(name="ps_s", bufs=2, space="PSUM"))
        ps_nm = ctx.enter_context(tc.tile_pool(name="ps_nm", bufs=2, space="PSUM"))
        ps_i = ctx.enter_context(tc.tile_pool(name="ps_i", bufs=1, space="PSUM"))
        zpool = ctx.enter_context(tc.tile_pool(name="zpool", bufs=3))

        # ---- constants ----
        tri = const.tile([P, P], f32r)
        nc.sync.dma_start(out=tri, in_=tri_d[:, :])
        ones2r = const.tile([P, 2], f32r)
        nc.sync.dma_start(out=ones2r, in_=ones2r_d[:, :])
        ones2b = const.tile([P, 2], bf16)
        nc.sync.dma_start(out=ones2b, in_=ones2b_d[:, :])
        onesb = const.tile([1, P], f32r)
        nc.sync.dma_start(out=onesb, in_=onesb_d[:, :])
        invcnt = const.tile([P, ROWS], f32)
        nc.sync.dma_start(out=invcnt, in_=invcnt_d[:, :])
        w_sb = const.tile([P, NH], f32)
        for h in range(NH):
            nc.sync.dma_start(out=w_sb[:, h : h + 1], in_=wvec[h * P : (h + 1) * P, 0:1])
        if with_bias:
            b_sb = const.tile([P, NH], f32)
            for h in range(NH):
                nc.sync.dma_start(
                    out=b_sb[:, h : h + 1], in_=bvec[h * P : (h + 1) * P, 0:1]
                )
        eps_sb = const.tile([P, 1], f32)
        nc.vector.memset(eps_sb, EPS)

        # ---- persistent stat-layout surfaces ----
        s_re = persist.tile([P, ROWS], f32)     # channel sums -> prefix sums
        sq_re = persist.tile([P, ROWS], f32)
        mean_t = persist.tile([P, ROWS], f32)
        ex2_t = persist.tile([P, ROWS], f32)    # E[x^2] -> var
        msq_t = persist.tile([P, ROWS], f32)    # mean^2 -> ln(var+eps)
        istd_t = persist.tile([P, ROWS], f32)
        nm_t = persist.tile([P, ROWS], f32)     # -mean
        st_sb = persist.tile([P, 2], f32r)      # chunk totals (s, sq)
        nc.sync.dma_start(out=st_sb, in_=zeros2_d[:, :])

        def phase_a(tix, x_t):
            """Load io-tile `tix` (2000 cols), compute channel sums/sumsq into
            stat-layout rows 16*tix .. 16*tix+16."""
            t0 = tix * CHUNK
            for h in range(NH):
                nc.sync.dma_start(
                    out=x_t[:, h, :],
                    in_=x[h * P : (h + 1) * P, t0 : t0 + CHUNK].bitcast(f32r),
                )
            xx0 = sqpool.tile([P, CHUNK], bf16, tag="xx0", name="xx0")
            nc.scalar.activation(xx0, x_t[:, 0, :].bitcast(f32), ACTF.Square)
            xx1 = sqpool.tile([P, CHUNK], bf16, tag="xx1", name="xx1")
            nc.gpsimd.tensor_tensor(
                xx1, x_t[:, 1, :].bitcast(f32), x_t[:, 1, :].bitcast(f32), ALU.mult
            )

            for a in range(NPB):
                cs = slice(a * PB, (a + 1) * PB)
                sps = ps_s.tile([2, 512], f32, tag="stat", name="sps")
                nc.tensor.matmul(
                    sps[0:2, 0:PB], ones2r, x_t[:, 0, cs], start=True, stop=False
                )
                nc.tensor.matmul(
                    sps[0:2, 0:PB], ones2r, x_t[:, 1, cs], start=False, stop=True
                )
                qps = ps_s.tile([2, 512], f32, tag="stat", name="qps")
                nc.tensor.matmul(
                    qps[0:2, 0:PB], ones2b, xx0[:, cs], start=True, stop=False
                )
                nc.tensor.matmul(
                    qps[0:2, 0:PB], ones2b, xx1[:, cs], start=False, stop=True
                )
                srow = erow.tile([1, PB], f32, tag="erow", name="srow")
                nc.scalar.copy(srow, sps[0:1, 0:PB])
                qrow = erow.tile([1, PB], f32, tag="erow", name="qrow")
                nc.scalar.copy(qrow, qps[0:1, 0:PB])
                # rows 16*tix+4a .. +4 of the stat layout (t = 125*p + i)
                rp = 16 * tix + 4 * a
                nc.sync.dma_start(out=s_re[rp : rp + 4, :], in_=srow)
                nc.sync.dma_start(out=sq_re[rp : rp + 4, :], in_=qrow)

        def stats(sc):
            """Prefix sums + mean/istd for stat-layout rows 32*sc .. 32*sc+32."""
            sl = slice(32 * sc, 32 * sc + 32)
            nc.vector.tensor_tensor_scan(
                out=s_re[sl, :], data0=s_re[sl, :], data1=s_re[sl, :],
                initial=0.0, op0=ALU.add, op1=ALU.bypass,
            )
            nc.vector.tensor_tensor_scan(
                out=sq_re[sl, :], data0=sq_re[sl, :], data1=sq_re[sl, :],
                initial=0.0, op0=ALU.add, op1=ALU.bypass,
            )
            nc.vector.tensor_copy(st_sb[sl, 0:1], s_re[sl, ROWS - 1 : ROWS])
            nc.vector.tensor_copy(st_sb[sl, 1:2], sq_re[sl, ROWS - 1 : ROWS])
            offps = ps_s.tile([P, 2], f32, tag="stat", name="offps")
            nc.tensor.matmul(offps, tri, st_sb, start=True, stop=True)

            nc.vector.scalar_tensor_tensor(
                out=mean_t[sl, :], in0=s_re[sl, :], scalar=offps[sl, 0:1],
                in1=invcnt[sl, :], op0=ALU.add, op1=ALU.mult,
            )
            nc.vector.scalar_tensor_tensor(
                out=ex2_t[sl, :], in0=sq_re[sl, :], scalar=offps[sl, 1:2],
                in1=invcnt[sl, :], op0=ALU.add, op1=ALU.mult,
            )
            nc.gpsimd.tensor_scalar_mul(nm_t[sl, :], mean_t[sl, :], -1.0)
            nc.gpsimd.tensor_tensor(msq_t[sl, :], mean_t[sl, :], mean_t[sl, :], ALU.mult)
            nc.gpsimd.tensor_tensor(ex2_t[sl, :], ex2_t[sl, :], msq_t[sl, :], ALU.subtract)
            # istd = 1 / sqrt(var + eps)  (Sqrt keeps the ACT table set stable;
            # reciprocal_approx_fast is ~18 bits, far above the fp32r noise)
            nc.scalar.activation(
                msq_t[sl, :], ex2_t[sl, :], ACTF.Sqrt, bias=eps_sb[sl, :], scale=1.0
            )
            nc.vector.reciprocal(out=istd_t[sl, :], in_=msq_t[sl, :])

        def phase_c(tix, x_t):
            """Normalize io-tile `tix` and store it."""
            t0 = tix * CHUNK
            rsl = slice(16 * tix, 16 * tix + 16)
            nm_row = brow.tile([1, CHUNK], f32r, tag="brow", name="nm_row")
            nc.sync.dma_start(out=nm_row, in_=nm_t[rsl, :].bitcast(f32r))
            istd_row = brow.tile([1, CHUNK], f32r, tag="brow", name="istd_row")
            nc.sync.dma_start(out=istd_row, in_=istd_t[rsl, :].bitcast(f32r))

            for half in range(2):  # half-tiles of 1000 columns
                nm_ps = ps_nm.tile([P, 2, 512], f32, tag="nm", name="nm_ps")
                ibc = ps_i.tile([P, 2, 512], f32, tag="ibc_ps", name="ibc")
                for j in range(2):
                    a = 2 * half + j
                    cs = slice(a * PB, (a + 1) * PB)
                    nc.tensor.matmul(
                        nm_ps[:, j, 0:PB], onesb, nm_row[0:1, cs],
                        start=True, stop=True,
                    )
                    nc.tensor.matmul(
                        ibc[:, j, 0:PB], onesb, istd_row[0:1, cs], start=True, stop=True
                    )
                ibc_sb = ibcsb.tile([P, 2, 512], f32, tag="ibc", name="ibc_sb")
                nc.scalar.copy(ibc_sb[:, :, 0:PB], ibc[:, :, 0:PB])
                for h in range(NH):
                    x_ap = x_t[:, h, half * 1000 : (half + 1) * 1000].bitcast(
                        f32
                    ).rearrange("p (j n) -> p j n", j=2)
                    # z = x - mean  (one DVE op; nm_ps is the -mean broadcast)
                    z_sb = zpool.tile([P, 2, 512], f32, tag="z", name="z_sb")
                    nc.vector.scalar_tensor_tensor(
                        out=z_sb[:, :, 0:PB], in0=nm_ps[:, :, 0:PB], scalar=1.0,
                        in1=x_ap, op0=ALU.mult, op1=ALU.add,
                    )
                    # y = (z * w) * istd
                    y_t = ypool.tile([P, 2, 512], f32, tag="y", name="y_t")
                    nc.vector.scalar_tensor_tensor(
                        out=y_t[:, :, 0:PB], in0=z_sb[:, :, 0:PB],
                        scalar=w_sb[:, h : h + 1], in1=ibc_sb[:, :, 0:PB],
                        op0=ALU.mult, op1=ALU.mult,
                    )
                    if with_bias:
                        nc.vector.tensor_scalar_add(
                            out=y_t[:, :, 0:PB], in0=y_t[:, :, 0:PB],
                            scalar1=b_sb[:, h : h + 1],
                        )
                    nc.sync.dma_start(
                        out=y[h * P : (h + 1) * P,
                              t0 + half * 1000 : t0 + (half + 1) * 1000],
                        in_=y_t[:, :, 0:PB],
                    )

        for sc in range(NCHUNK // 2):  # stats chunks of 4000 cols
            tiles = []
            for k in range(2):
                x_t = xpool.tile([P, NH, CHUNK], f32r, tag="x", name="x_t")
                phase_a(2 * sc + k, x_t)
                tiles.append(x_t)
            stats(sc)
            for k in range(2):
                phase_c(2 * sc + k, tiles[k])
    nc.compile()
    return nc


def _consts():
    iden = np.eye(P, dtype=np.float32)
    tri = np.triu(np.ones((P, P), dtype=np.float32), k=1)  # tri[k,m]=1 iff k<m
    ones2 = np.ones((P, 2), dtype=np.float32)
    onesb = np.ones((1, P), dtype=np.float32)
    t_idx = (125 * np.arange(P, dtype=np.float64)[:, None]
             + np.arange(ROWS, dtype=np.float64)[None, :])
    invcnt = (1.0 / (C * (t_idx + 1.0))).astype(np.float32)
    return {"iden": iden, "tri": tri, "ones2r": ones2,
            "ones2b": ones2.astype(ml_dtypes.bfloat16), "onesb": onesb,
            "zeros2": np.zeros((P, 2), dtype=np.float32), "invcnt": invcnt}


def _get_nc(with_bias: bool):
    key = ("nc", with_bias)
    if key not in _cached:
        _cached[key] = _build_nc(with_bias)
    return _cached[key]


def _run(x, weight, bias, trace=False):
    from concourse.bass_utils import run_bass_kernel_spmd

    x = np.ascontiguousarray(np.asarray(x, dtype=np.float32))
    weight = np.asarray(weight, dtype=np.float32).reshape(C, 1)
    bias = np.asarray(bias, dtype=np.float32).reshape(C, 1)
    with_bias = bool(np.any(bias))
    nc = _get_nc(with_bias)

    consts = _consts()
    in_maps = []
    for b in range(B):
        m = {"x": np.ascontiguousarray(x[b]), "wvec": weight}
        if with_bias:
            m["bvec"] = bias
        m.update(consts)
        in_maps.append(m)

    res = run_bass_kernel_spmd(nc, in_maps, core_ids=list(range(B)), trace=trace)
    y = np.stack([r["y"] for r in res.results], axis=0)
    return y, res


def kernel(x, weight, bias):
    y, _ = _run(x, weight, bias, trace=False)
    return y
